# revision 13
# baseline (speedup 1.0000x reference)
"""AlphaFold-style gated MSA attention on 8 Trainium2 NeuronCores.

Batch-sharded (128 batches -> 16 per core). Full inputs in, full output out.

Math per batch b (reference):
  q = (q_data @ Wq) * hk^-0.5          [Q, H, 32]
  k = m_data @ Wk ; v = m_data @ Wv    [K, H, 32]
  S[h] = q_h k_h^T + bias[b] + nb[h]   [H, Q, K]
  w = softmax(S, axis=-1)
  wa = w @ v                            [Q, H, 32]
  gate = sigmoid(q_data @ Wg + gb)
  out = (wa * gate).reshape(Q, 256) @ Wo + o_bias

Device-side formulation (per core):
  - All projections run as fp8e4 DoubleRow matmuls (contraction 256 = 2
    k-tiles of 128 in one instruction at 0.5 cycles/row).  Host pre-scales
    qd/md by 1/8 and the weights by 64 to center fp8 magnitudes; the PSUM
    evacuations scale by 1/8 (DVE tensor_scalar) to restore true values.
  - The additive biases are folded into the S PSUM accumulation: host ships
    fused[b,h] = (bias[b] + nb[h])^T in fp8, and per (head, k-chunk) a
    DoubleRow identity matmul (lhsT = [I|0]) adds it to S^T.  exp() then
    reads the complete logits from PSUM and writes the softmax numerator
    directly as fp8 (ACT bias applies a -2 shift to keep exp() inside the
    fp8e4 range; the shift cancels in the softmax normalization).
  - The V matmul uses lhsT = [v_h | 2.0] packed per head so PSUM row 32
    accumulates 2*sum_k w (softmax denominators for free), with kc0/kc1 as
    one fp8 DoubleRow matmul and kc2 plain fp8.
  - The per-pair psW tiles are evacuated into one waBIG tile; a single
    descriptor-merged DMA gathers the 8 denominator rows and two DMAs
    rearrange the head blocks into gate-aligned waA tiles (engines cannot
    shift partitions; DMA can, and merging keeps HWDGE occupancy low).
  - Normalization, gating and the output projection follow the rank-1
    tricks of the bf16 version: an indicator matmul broadcasts 1/(2*sum)
    across each head's 32 partitions, gate fuses via scalar_tensor_tensor,
    and o_bias rides a ones-row rank-1 matmul into the PSUM group.
"""

import os
import sys

sys.path.insert(0, "/opt/trn_rl_repo")

import numpy as np
import ml_dtypes
from contextlib import ExitStack

import concourse.bass as bass  # noqa: F401  (engine types)
import concourse.bacc as bacc
import concourse.mybir as mybir
import concourse.tile as tile

BF16 = ml_dtypes.bfloat16
F8 = ml_dtypes.float8_e4m3fn

NUM_CORES = 8
B, Q, K, A = 128, 384, 384, 256
H, HD = 8, 32  # heads, head dim
OUT = 256
BPC = B // NUM_CORES  # batches per core

SHIFT = 3.0  # exp(logit - SHIFT): keeps fp8e4 w below saturation

DR = mybir.MatmulPerfMode.DoubleRow


def _env(name, default):
    return os.environ.get(name, default)


def _build_body(ctx, tc, io, bpc):
    nc = tc.nc
    f32, bf, f8 = mybir.dt.float32, mybir.dt.bfloat16, mybir.dt.float8e4
    Exp = mybir.ActivationFunctionType.Exp
    Tanh = mybir.ActivationFunctionType.Tanh
    MUL, ADD = mybir.AluOpType.mult, mybir.AluOpType.add

    import bass_rust as _br
    from concourse.tile_rust import add_dep_helper as _adh

    const = ctx.enter_context(tc.tile_pool(name="const", bufs=1))
    lp = ctx.enter_context(tc.tile_pool(name="loads", bufs=int(_env("LP_BUFS", "3"))))
    pp = ctx.enter_context(tc.tile_pool(name="proj", bufs=int(_env("PP_BUFS", "3"))))
    wp = ctx.enter_context(tc.tile_pool(name="work", bufs=int(_env("WP_BUFS", "4"))))
    wap = ctx.enter_context(tc.tile_pool(name="wa", bufs=int(_env("WA_BUFS", "2"))))
    gp = ctx.enter_context(tc.tile_pool(name="gating", bufs=int(_env("GP_BUFS", "3"))))
    outp = ctx.enter_context(tc.tile_pool(name="outp", bufs=3))
    Sp = ctx.enter_context(tc.tile_pool(name="psum_S", bufs=2, space="PSUM"))
    sp = ctx.enter_context(tc.tile_pool(name="psum_sm", bufs=2, space="PSUM"))

    VV_BUFS = int(_env("VV_BUFS", "3"))

    # ---- resident constants ----
    w_sb = {}
    for name in ("wq", "wg", "wk"):
        w_sb[name] = const.tile([128, 2, 256], f8, tag=name, name=name)
        nc.sync.dma_start(w_sb[name][:], io[name])
    for name in ("wv",):
        w_sb[name] = const.tile([128, 2, 256], bf, tag=name, name=name)
        nc.sync.dma_start(w_sb[name][:], io[name])
    w_sb["wo"] = const.tile([128, 2, 256], bf, tag="wo", name="wo")
    nc.sync.dma_start(w_sb["wo"][:], io["wo"])
    obias_row = const.tile([1, OUT], bf, tag="obias_row")
    nc.sync.dma_start(obias_row[:], io["obias_bf"])
    ones_row = const.tile([1, 128], bf, tag="ones_row")
    nc.sync.dma_start(ones_row[:], io["ind"][127:128, 0:128])
    gbh_sb = const.tile([128, 2], f32, tag="gbh")
    nc.sync.dma_start(gbh_sb[:], io["gbh"])
    shift_sb = const.tile([128, 1], f32, tag="shift")
    nc.gpsimd.memset(shift_sb[:], -SHIFT)
    # dummy activation early: pulls the ACT table load off the critical path
    warm = const.tile([128, 2], f32, tag="warm")
    nc.scalar.activation(warm[:], gbh_sb[:], Exp)
    ind_sb = const.tile([128, 256], bf, tag="ind")
    nc.sync.dma_start(ind_sb[:], io["ind"])
    # [I | 0] for the DoubleRow bias fold
    id2 = const.tile([128, 2, 128], f8, tag="id2")
    id2_dma = nc.sync.dma_start(id2[:], io["id2"])

    # ---- stable-buffer init: vv tiles carry 2.0 in the sum column slots ----
    vv_init = []
    for i in range(VV_BUFS):
        t = pp.tile([128, 3, 320], bf, tag="vv", name=f"vv_init{i}", bufs=VV_BUFS)
        nc.gpsimd.memset(t[:], 2.0)
        vv_init.append(t)
    # kTI: slot 0 = I (for the S+lo DoubleRow), slots 1+3h+kc = per-(h,kc)
    # zero-padded kT slabs (head rows live at 32*hh, zeros elsewhere)
    KTI_BUFS = int(_env("KTI_BUFS", "2"))
    kti_init = []
    for i in range(KTI_BUFS):
        t = pp.tile([128, 25, 128], f8, tag="kTI", name=f"kTI_init{i}", bufs=KTI_BUFS)
        nc.gpsimd.memset(t[:], 0.0)
        kti_init.append((t, nc.sync.dma_start(t[:, 0, :], io["id2"][:, 0, :])))
    # nbb: slots 0-2 = bias_hi(b) (per-batch DMA), 3-26 = nb_hi (resident)
    nbb_init = []
    for i in range(2):
        t = pp.tile([128, 27, Q], f8, tag="nbb", name=f"nbb_init{i}", bufs=2)
        nbb_init.append((t, nc.sync.dma_start(t[:, 3:27, :], io["nbhi"])))

    def emit_loads(b):
        # ld: 0-1 qd8, 2-3 md8, 4-27 lo_comb(h,kc), 28-29 qT8 (evac space)
        ld = lp.tile([128, 30, Q], f8, tag="ld", name=f"ld_{b}")
        nc.sync.dma_start(ld[:, 0:28, :], io["inT"][b])
        ldb = lp.tile([128, 2, Q], bf, tag="ldb", name=f"ldb_{b}")
        nc.sync.dma_start(ldb[:], io["inTb"][b])
        nbb = pp.tile([128, 27, Q], f8, tag="nbb", name=f"nbb_{b}", bufs=2)
        nbd = nc.sync.dma_start(nbb[:, 0:3, :], io["bhi"][b])
        if b < 2:
            _adh(nbd.ins, nbb_init[b % 2][1].ins, reason="nbhi resident before bias_hi")
        return ld, ldb, nbb

    def emit_proj(b, ld, ldb, nbb):
        # q/gate: fp8 DoubleRow (evac rescales 1/8); k/v: bf16 (accuracy).
        gt = pp.tile([128, 2, Q], bf, tag="gt", name=f"gt_{b}")
        vv = pp.tile([128, 3, 320], bf, tag="vv", name=f"vv_{b}", bufs=VV_BUFS)
        kTI = pp.tile([128, 25, 128], f8, tag="kTI", name=f"kTI_{b}", bufs=KTI_BUFS)
        if b < KTI_BUFS:
            pass  # zeros + I come from the startup instances on this buffer
        qd = ld[:, 0:2, :]
        md = ld[:, 2:4, :]
        for j in range(2):
            ps = sp.tile([128, 512], f32, tag="sm", name=f"psq{j}_{b}")
            nc.tensor.matmul(
                ps[:, :Q], w_sb["wq"][:, :, 128 * j : 128 * (j + 1)], qd,
                start=True, stop=True, perf_mode=DR,
            )
            # psQ = 8*q ; store qT8 = 4*q in the ld rhs slot
            nc.vector.tensor_scalar_mul(ld[:, 28 + j, :], ps[:, :Q], 0.5)
            ps = sp.tile([128, 512], f32, tag="sm", name=f"psk{j}_{b}")
            nc.tensor.matmul(
                ps[:, :Q], w_sb["wk"][:, :, 128 * j : 128 * (j + 1)], md,
                start=True, stop=True, perf_mode=DR,
            )
            # psK = 8*k ; DVE rescales to k/4 in SBUF, Pool scatters into
            # the zero-padded slabs (GPSIMD cannot read PSUM)
            kq = pp.tile([128, 2, Q], bf, tag="kq", name=f"kq{j}_{b}")
            nc.vector.tensor_scalar_mul(kq[:, j, :], ps[:, :Q], 0.03125)
            for hh in range(4):
                h = 4 * j + hh
                nc.gpsimd.tensor_copy(
                    kTI[32 * hh : 32 * hh + 32, 1 + 3 * h : 4 + 3 * h, :],
                    kq[32 * hh : 32 * hh + 32, j, :].rearrange(
                        "p (c k) -> p c k", k=128
                    ),
                )
            ps = sp.tile([128, 512], f32, tag="sm", name=f"psg{j}_{b}")
            nc.tensor.matmul(
                ps[:, :Q], w_sb["wg"][:, :, 128 * j : 128 * (j + 1)], qd,
                start=True, stop=True, perf_mode=DR,
            )
            nc.scalar.activation(
                gt[:, j, :], ps[:, :Q], Tanh, bias=gbh_sb[:, j : j + 1], scale=0.0625
            )
        for kc in range(3):
            ps = sp.tile([128, 512], f32, tag="sm", name=f"psv{kc}_{b}")
            for a in range(2):
                nc.tensor.matmul(
                    ps[:, :256],
                    ldb[:, a, 128 * kc : 128 * (kc + 1)],
                    w_sb["wv"][:, a, :],
                    start=(a == 0), stop=(a == 1),
                )
            nc.vector.tensor_copy(
                vv[:, kc, 0:264].rearrange("p (h c) -> p h c", c=33)[:, :, 0:32],
                ps[:, :256].rearrange("p (h c) -> p h c", c=32),
            )
        sums_bf = gp.tile([8, Q], bf, tag="sums_bf", name=f"sums_bf_{b}")
        waBIG = wap.tile([128, 4, Q], bf, tag="waBIG", name=f"waBIG_{b}")
        waA = [
            gp.tile([128, Q], bf, tag=f"waA{j}", name=f"waA{j}_{b}") for j in range(2)
        ]
        return dict(
            ld=ld, kTI=kTI, nbb=nbb, gt=gt, vv=vv,
            sums_bf=sums_bf, waBIG=waBIG, waA=waA,
            wa_evac=[None] * 4, psW=None,
        )

    def emit_heads(b, st, first=False):
        ld, kTI, nbb = st["ld"], st["kTI"], st["nbb"]
        vv, waBIG = st["vv"], st["waBIG"]
        for h in range(8):
            j, hh, p, pw = h // 4, h % 4, h // 2, h % 2
            psS = Sp.tile([128, 1536], f32, tag="S", name=f"psS{h}_{b}")
            for kc in range(3):
                s = 3 * h + kc
                # DR1: psS = I*lo_comb + slab.T*qT8  (= lo + S^T)
                sm = nc.tensor.matmul(
                    psS[:, 512 * kc : 512 * kc + Q],
                    kTI[:, 0 : 2 + s : 1 + s, :],
                    ld[:, 4 + s : 29 + j : 24 + j - s, :],
                    start=True, stop=False, perf_mode=DR,
                )
                if first and h == 0:
                    _adh(sm.ins, kti_init[0][1].ins, reason="I before first S-DR")
                # DR2: psS += bias_hi(kc) + nb_hi(h,kc)
                fm = nc.tensor.matmul(
                    psS[:, 512 * kc : 512 * kc + Q],
                    id2[:],
                    nbb[:, kc : 4 + s : 3 + 3 * h, :],
                    start=False, stop=True, perf_mode=DR,
                )
                if first and h == 0:
                    _adh(fm.ins, id2_dma.ins, reason="id2 load before first fold")
            sview = psS[:].rearrange("p (c x) -> p c x", x=512)[:, :, :Q]
            w4 = wp.tile([128, 3, Q], bf, tag="w4", name=f"w4_{h}_{b}")
            nc.scalar.activation(w4[:], sview, Exp, bias=shift_sb[:])
            if pw == 0:
                st["psW"] = sp.tile([128, 512], f32, tag="sm", name=f"psW{h}_{b}")
            psW = st["psW"]
            for kc in range(3):
                nc.tensor.matmul(
                    psW[64 * pw : 64 * pw + 64, :Q],
                    vv[:, kc, 33 * h : 33 * h + 64],
                    w4[:, kc, :],
                    start=(kc == 0), stop=(kc == 2),
                )
            if pw == 1:
                ev = nc.vector.tensor_copy(waBIG[:, p, :], psW[:, :Q])
                st["wa_evac"][p] = ev

    # raw-AP gather bookkeeping for WAR insurance on waBIG reuse
    last_gathers = {}

    def emit_tail(b, st):
        waBIG, sums_bf, waA, gt = st["waBIG"], st["sums_bf"], st["waA"], st["gt"]
        evacs = st["wa_evac"]
        parity = b % int(_env("WA_BUFS", "2"))
        # one DMA: the 8 denominator rows (2*sum) -> sums_bf [8, Q].
        # Row order is (r, p): row i holds head 2*(i%4) + i//4; the host ind
        # matrix is permuted to match.
        src = _br.AP(
            waBIG.tensor,
            waBIG.offset + 32 * 4 * Q,
            [[64 * 4 * Q, 2], [Q, 4], [1, Q]],
        )
        dst = _br.AP(sums_bf.tensor, sums_bf.offset, [[Q, 8], [1, Q]])
        d = nc.sync.dma_start(dst, src)
        for ev in evacs:
            _adh(d.ins, ev.ins, reason="sums gather reads waBIG")
        gathers = [d]
        # cast + reciprocal + bf16 rebroadcast source
        sums_f = gp.tile([8, Q], f32, tag="sums_f", name=f"sums_f_{b}")
        sc = nc.gpsimd.tensor_copy(sums_f[:], sums_bf[:])
        _adh(sc.ins, d.ins, reason="sums_bf filled by gather")
        rec = gp.tile([8, Q], f32, tag="rec", name=f"rec_{b}")
        nc.vector.reciprocal_approx_fast(rec[:], sums_f[:])
        recb = gp.tile([8, Q], bf, tag="recb", name=f"recb_{b}")
        nc.gpsimd.tensor_copy(recb[:], rec[:])
        # head blocks into gate-aligned waA tiles (tile-slice DMAs; partition
        # shifts are DMA-only).  head h: waBIG[64*(h%2):+32, h//2] ->
        # waA[h//4][32*(h%4):+32]
        wadmas = []
        for h in range(8):
            j, hh, p, r = h // 4, h % 4, h // 2, h % 2
            dj = nc.sync.dma_start(
                waA[j][32 * hh : 32 * hh + 32, :],
                waBIG[64 * r : 64 * r + 32, p, :],
            )
            wadmas.append(dj)
            gathers.append(dj)
        last_gathers[parity] = gathers

        ga_tiles = []
        for j in range(2):
            psR = sp.tile([128, 512], f32, tag="sm", name=f"psR{j}_{b}")
            nc.tensor.matmul(
                psR[:, :Q], ind_sb[0:8, 128 * j : 128 * (j + 1)], recb[:],
                start=True, stop=True,
            )
            g2 = gp.tile([128, Q], bf, tag="g2", name=f"g2{j}_{b}")
            nc.vector.scalar_tensor_tensor(
                g2[:], gt[:, j, :], 1.0, psR[:, :Q], op0=ADD, op1=MUL
            )
            ga = gp.tile([128, Q], bf, tag="ga", name=f"ga{j}_{b}")
            gm = nc.gpsimd.tensor_tensor(ga[:], waA[j][:], g2[:], op=MUL)
            ga_tiles.append(ga)
        ob = outp.tile([128, 3, OUT], f32, tag="ob", name=f"ob_{b}")
        for qc in range(3):
            psO = sp.tile([128, 512], f32, tag="sm", name=f"psO{qc}_{b}")
            for j in range(2):
                nc.tensor.matmul(
                    psO[:, :OUT],
                    ga_tiles[j][:, 128 * qc : 128 * (qc + 1)],
                    w_sb["wo"][:, j, :],
                    start=(j == 0), stop=False,
                )
            nc.tensor.matmul(
                psO[:, :OUT], ones_row[:], obias_row[:], start=False, stop=True
            )
            nc.vector.tensor_copy(ob[:, qc, :], psO[:, :OUT])
        nc.sync.dma_start(io["out"][b].rearrange("(c p) o -> p c o", p=128), ob[:])

    def guard_evacs(st, b):
        # WAR insurance: this batch's waBIG writes wait for the gathers that
        # read the buffer two batches ago (raw-AP reads are invisible to the
        # tile tracker).
        parity = b % int(_env("WA_BUFS", "2"))
        old = last_gathers.get(parity)
        if old:
            for ev in st["wa_evac"]:
                for g in old:
                    _adh(ev.ins, g.ins, reason="waBIG reuse after raw gather")

    # Software pipeline: loads+projections of batch b, then the latency-heavy
    # tail of batch b-1 (overlapping this batch's heads).
    prev = None
    for b in range(bpc):
        ld, ldb, nbb = emit_loads(b)
        st = emit_proj(b, ld, ldb, nbb)
        if prev is not None:
            emit_tail(b - 1, prev)
        emit_heads(b, st, first=(b == 0))
        guard_evacs(st, b)
        prev = st
    emit_tail(bpc - 1, prev)


def build(bpc=BPC):
    nc = bacc.Bacc(
        "TRN2",
        target_bir_lowering=False,
        debug=False,
        enable_asserts=False,
        num_devices=NUM_CORES,
    )
    f32, bf, f8 = mybir.dt.float32, mybir.dt.bfloat16, mybir.dt.float8e4
    io = {
        "inT": nc.dram_tensor("inT", [bpc, 128, 28, Q], f8, kind="ExternalInput").ap(),
        "inTb": nc.dram_tensor("inTb", [bpc, 128, 2, Q], bf, kind="ExternalInput").ap(),
        "wq": nc.dram_tensor("wq", [128, 2, 256], f8, kind="ExternalInput").ap(),
        "wk": nc.dram_tensor("wk", [128, 2, 256], f8, kind="ExternalInput").ap(),
        "wv": nc.dram_tensor("wv", [128, 2, 256], bf, kind="ExternalInput").ap(),
        "wg": nc.dram_tensor("wg", [128, 2, 256], f8, kind="ExternalInput").ap(),
        "wo": nc.dram_tensor("wo", [128, 2, 256], bf, kind="ExternalInput").ap(),
        "obias_bf": nc.dram_tensor("obias_bf", [1, OUT], bf, kind="ExternalInput").ap(),
        "gbh": nc.dram_tensor("gbh", [128, 2], f32, kind="ExternalInput").ap(),
        "ind": nc.dram_tensor("ind", [128, 256], bf, kind="ExternalInput").ap(),
        "id2": nc.dram_tensor("id2", [128, 2, 128], f8, kind="ExternalInput").ap(),
        "nbhi": nc.dram_tensor("nbhi", [128, 24, Q], f8, kind="ExternalInput").ap(),
        "bhi": nc.dram_tensor("bhi", [bpc, 128, 3, Q], f8, kind="ExternalInput").ap(),
        "out": nc.dram_tensor("out", [bpc, Q, OUT], f32, kind="ExternalOutput").ap(),
    }
    with tile.TileContext(nc) as tc:
        with ExitStack() as ctx:
            _build_body(ctx, tc, io, bpc)
    nc.compile()
    return nc


def _prep_inputs(
    q_data,
    m_data,
    bias,
    nonbatched_bias,
    q_weights,
    k_weights,
    v_weights,
    o_weights,
    o_bias,
    gating_w,
    gating_b,
):
    """Host-side preprocessing into the DMA-friendly device layouts."""
    scale = q_weights.shape[-1] ** -0.5

    def featT(x, s):  # [B, S, A] -> [B, 128, A//128, S] scaled
        b, sl, a = x.shape
        t = x.transpose(0, 2, 1).reshape(b, a // 128, 128, sl).transpose(0, 2, 1, 3)
        return np.ascontiguousarray((t * s).astype(F8))

    qdT = featT(q_data, 0.125)  # [B, 128, 2, Q]

    def featTb(x):  # [B, S, A] -> [B, 128, A//128, S] bf16
        b, sl, a = x.shape
        t = x.transpose(0, 2, 1).reshape(b, a // 128, 128, sl).transpose(0, 2, 1, 3)
        return np.ascontiguousarray(t.astype(BF16))

    mdTb = featTb(m_data)

    # fused[b, h] = (bias[b] + nb[h])^T in chunk layout [128, 8, 3, Q]
    biasT = bias[:, 0].transpose(0, 2, 1).astype(np.float32)  # [B, K, Q]
    nbT = nonbatched_bias.transpose(0, 2, 1).astype(np.float32)  # [H, K, Q]
    mdT8 = featT(m_data, 0.125)
    inT = np.empty((B, 128, 28, Q), dtype=F8)
    inT[:, :, 0:2, :] = qdT
    inT[:, :, 2:4, :] = mdT8
    # coarse halves: 0.5-granular, exactly representable in fp8e4
    nbhi_f = np.clip(np.round(nbT * 2.0) / 2.0, -8.0, 8.0)  # [H, K, Q]
    bhi_f = np.clip(np.round(biasT * 2.0) / 2.0, -8.0, 8.0)  # [B, K, Q]
    nbhi = np.ascontiguousarray(
        nbhi_f.reshape(H, 3, 128, Q).transpose(2, 0, 1, 3).reshape(128, 24, Q)
    ).astype(F8)
    bhi = np.ascontiguousarray(
        bhi_f.reshape(B, 3, 128, Q).transpose(0, 2, 1, 3)
    ).astype(F8)
    for b in range(B):
        lo = (biasT[b] - bhi_f[b])[None] + (nbT - nbhi_f)  # [H, K, Q]
        fc = lo.reshape(H, 3, 128, Q).transpose(2, 0, 1, 3)
        inT[b, :, 4:28, :] = fc.reshape(128, 24, Q).astype(F8)

    def wmat(w, s):  # [A, H, hd] -> [128, 2, 256]
        m = (w.reshape(A, H * HD) * s).astype(F8)
        return np.ascontiguousarray(m.reshape(2, 128, 256).transpose(1, 0, 2))

    def wmatb(w):  # [A, H, hd] -> [128, 2, 256] bf16
        m = w.reshape(A, H * HD).astype(BF16)
        return np.ascontiguousarray(m.reshape(2, 128, 256).transpose(1, 0, 2))

    wq = wmat(q_weights, 64.0 * scale)
    wk = wmat(k_weights, 64.0)
    wv = wmatb(v_weights)
    wg = wmat(gating_w, 64.0)
    wo = np.ascontiguousarray(
        o_weights.reshape(256, 256).astype(BF16).reshape(2, 128, 256).transpose(1, 0, 2)
    )
    obias_bf = np.ascontiguousarray(o_bias.astype(BF16).reshape(1, OUT))
    gbh = np.ascontiguousarray(
        (0.5 * gating_b.reshape(H * HD).astype(np.float32)).reshape(2, 128).T
    )
    ind = np.zeros((128, 256), dtype=BF16)
    # sums_bf row i holds head 2*(i%4) + i//4 (gather iterates (r, p))
    for i in range(8):
        h = 2 * (i % 4) + i // 4
        ind[i, 32 * h : 32 * (h + 1)] = 1.0
    ind[127, :] = 1.0  # ones row for the o_bias rank-1 matmul
    id2 = np.zeros((128, 2, 128), dtype=F8)
    id2[:, 0, :] = np.eye(128, dtype=np.float32).astype(F8)
    id2[:, 1, :] = id2[:, 0, :]
    return dict(
        inT=inT, inTb=mdTb, wq=wq, wk=wk, wv=wv, wg=wg, wo=wo,
        obias_bf=obias_bf, gbh=gbh, ind=ind, id2=id2, nbhi=nbhi, bhi=bhi,
    )


_NC_CACHE = {}


def kernel(**inputs):
    from concourse.bass_utils import run_bass_kernel_spmd

    full = _prep_inputs(**{k: np.asarray(v) for k, v in inputs.items()})
    if BPC not in _NC_CACHE:
        _NC_CACHE[BPC] = build(BPC)
    nc = _NC_CACHE[BPC]

    shared = {
        k: full[k]
        for k in ("wq", "wk", "wv", "wg", "wo", "obias_bf", "gbh", "ind", "id2", "nbhi")
    }
    in_maps = []
    for c in range(NUM_CORES):
        sl = slice(c * BPC, (c + 1) * BPC)
        in_maps.append(dict(inT=full["inT"][sl], inTb=full["inTb"][sl], bhi=full["bhi"][sl], **shared))

    trace = bool(int(os.environ.get("BASS_KERNEL_TRACE", "0")))
    if trace:
        try:
            from antenv.axon_hooks import get_axon_ntff_profile_hook  # noqa: F401
        except Exception:
            trace = False
    import time

    t0 = time.time()
    res = run_bass_kernel_spmd(
        nc, in_maps, core_ids=list(range(NUM_CORES)), trace=trace
    )
    kernel.last_run_wall_s = time.time() - t0
    if trace and res.exec_time_ns is not None:
        print(f"HW exec time: {res.exec_time_ns} ns")
        kernel.last_exec_time_ns = res.exec_time_ns
    out = np.concatenate([r["out"] for r in res.results], axis=0)
    return out.astype(np.float32)


# revision 14
# speedup vs baseline: 1.1477x; 1.1477x over previous
"""AlphaFold-style gated MSA attention on 8 Trainium2 NeuronCores.

Batch-sharded (128 batches -> 16 per core). Full inputs in, full output out.

Math per batch b (reference):
  q = (q_data @ Wq) * hk^-0.5          [Q, H, 32]
  k = m_data @ Wk ; v = m_data @ Wv    [K, H, 32]
  S[h] = q_h k_h^T + bias[b] + nb[h]   [H, Q, K]
  w = softmax(S, axis=-1)
  wa = w @ v                            [Q, H, 32]
  gate = sigmoid(q_data @ Wg + gb)
  out = (wa * gate).reshape(Q, 256) @ Wo + o_bias

Device-side formulation (per core):
  - All projections run as fp8e4 DoubleRow matmuls (contraction 256 = 2
    k-tiles of 128 in one instruction at 0.5 cycles/row).  Host pre-scales
    qd/md by 1/8 and the weights by 64 to center fp8 magnitudes; the PSUM
    evacuations scale by 1/8 (DVE tensor_scalar) to restore true values.
  - The additive biases are folded into the S PSUM accumulation: host ships
    fused[b,h] = (bias[b] + nb[h])^T in fp8, and per (head, k-chunk) a
    DoubleRow identity matmul (lhsT = [I|0]) adds it to S^T.  exp() then
    reads the complete logits from PSUM and writes the softmax numerator
    directly as fp8 (ACT bias applies a -2 shift to keep exp() inside the
    fp8e4 range; the shift cancels in the softmax normalization).
  - The V matmul uses lhsT = [v_h | 2.0] packed per head so PSUM row 32
    accumulates 2*sum_k w (softmax denominators for free), with kc0/kc1 as
    one fp8 DoubleRow matmul and kc2 plain fp8.
  - The per-pair psW tiles are evacuated into one waBIG tile; a single
    descriptor-merged DMA gathers the 8 denominator rows and two DMAs
    rearrange the head blocks into gate-aligned waA tiles (engines cannot
    shift partitions; DMA can, and merging keeps HWDGE occupancy low).
  - Normalization, gating and the output projection follow the rank-1
    tricks of the bf16 version: an indicator matmul broadcasts 1/(2*sum)
    across each head's 32 partitions, gate fuses via scalar_tensor_tensor,
    and o_bias rides a ones-row rank-1 matmul into the PSUM group.
"""

import os
import sys

sys.path.insert(0, "/opt/trn_rl_repo")

import numpy as np
import ml_dtypes
from contextlib import ExitStack

import concourse.bass as bass  # noqa: F401  (engine types)
import concourse.bacc as bacc
import concourse.mybir as mybir
import concourse.tile as tile

BF16 = ml_dtypes.bfloat16
F8 = ml_dtypes.float8_e4m3fn

NUM_CORES = 8
B, Q, K, A = 128, 384, 384, 256
H, HD = 8, 32  # heads, head dim
OUT = 256
BPC = B // NUM_CORES  # batches per core

SHIFT = 3.0  # exp(logit - SHIFT): keeps fp8e4 w below saturation

DR = mybir.MatmulPerfMode.DoubleRow


def _env(name, default):
    return os.environ.get(name, default)


def _build_body(ctx, tc, io, bpc):
    nc = tc.nc
    f32, bf, f8 = mybir.dt.float32, mybir.dt.bfloat16, mybir.dt.float8e4
    Exp = mybir.ActivationFunctionType.Exp
    Tanh = mybir.ActivationFunctionType.Tanh
    MUL, ADD = mybir.AluOpType.mult, mybir.AluOpType.add

    import bass_rust as _br
    from concourse.tile_rust import add_dep_helper as _adh

    const = ctx.enter_context(tc.tile_pool(name="const", bufs=1))
    lp = ctx.enter_context(tc.tile_pool(name="loads", bufs=int(_env("LP_BUFS", "4"))))
    pp = ctx.enter_context(tc.tile_pool(name="proj", bufs=int(_env("PP_BUFS", "4"))))
    wp = ctx.enter_context(tc.tile_pool(name="work", bufs=int(_env("WP_BUFS", "4"))))
    wap = ctx.enter_context(tc.tile_pool(name="wa", bufs=int(_env("WA_BUFS", "4"))))
    gp = ctx.enter_context(tc.tile_pool(name="gating", bufs=int(_env("GP_BUFS", "4"))))
    outp = ctx.enter_context(tc.tile_pool(name="outp", bufs=3))
    Sp = ctx.enter_context(tc.tile_pool(name="psum_S", bufs=2, space="PSUM"))
    sp = ctx.enter_context(tc.tile_pool(name="psum_sm", bufs=2, space="PSUM"))

    VV_BUFS = int(_env("VV_BUFS", "3"))

    # ---- resident constants ----
    w_sb = {}
    for name in ("wq", "wg", "wk"):
        w_sb[name] = const.tile([128, 2, 256], f8, tag=name, name=name)
        nc.sync.dma_start(w_sb[name][:], io[name])
    for name in ("wv",):
        w_sb[name] = const.tile([128, 2, 256], bf, tag=name, name=name)
        nc.sync.dma_start(w_sb[name][:], io[name])
    w_sb["wo"] = const.tile([128, 2, 256], bf, tag="wo", name="wo")
    nc.sync.dma_start(w_sb["wo"][:], io["wo"])
    obias_row = const.tile([1, OUT], bf, tag="obias_row")
    nc.sync.dma_start(obias_row[:], io["obias_bf"])
    ones_row = const.tile([1, 128], bf, tag="ones_row")
    nc.sync.dma_start(ones_row[:], io["ind"][127:128, 0:128])
    gbh_sb = const.tile([128, 2], f32, tag="gbh")
    nc.sync.dma_start(gbh_sb[:], io["gbh"])
    shift_sb = const.tile([128, 1], f32, tag="shift")
    nc.gpsimd.memset(shift_sb[:], -SHIFT)
    # dummy activation early: pulls the ACT table load off the critical path
    warm = const.tile([128, 2], f32, tag="warm")
    nc.scalar.activation(warm[:], gbh_sb[:], Exp)
    ind_sb = const.tile([128, 256], bf, tag="ind")
    nc.sync.dma_start(ind_sb[:], io["ind"])
    # [I | 0] for the DoubleRow bias fold
    id2 = const.tile([128, 2, 128], f8, tag="id2")
    id2_dma = nc.sync.dma_start(id2[:], io["id2"])

    # ---- stable-buffer init: vv tiles carry 2.0 in the sum column slots ----
    vv_init = []
    for i in range(VV_BUFS):
        t = pp.tile([128, 3, 320], bf, tag="vv", name=f"vv_init{i}", bufs=VV_BUFS)
        nc.gpsimd.memset(t[:], 2.0)
        vv_init.append(t)
    # kTI: slot 0 = I (for the S+lo DoubleRow), slots 1+3h+kc = per-(h,kc)
    # zero-padded kT slabs (head rows live at 32*hh, zeros elsewhere)
    KTI_BUFS = int(_env("KTI_BUFS", "3"))
    kti_init = []
    for i in range(KTI_BUFS):
        t = pp.tile([128, 25, 128], f8, tag="kTI", name=f"kTI_init{i}", bufs=KTI_BUFS)
        nc.gpsimd.memset(t[:], 0.0)
        kti_init.append((t, nc.sync.dma_start(t[:, 0, :], io["id2"][:, 0, :])))
    # nbb: slots 0-2 = bias_hi(b) (per-batch DMA), 3-26 = nb_hi (resident)
    NBB_BUFS = int(_env("NBB_BUFS", "3"))
    nbb_init = []
    for i in range(NBB_BUFS):
        t = pp.tile([128, 27, Q], f8, tag="nbb", name=f"nbb_init{i}", bufs=NBB_BUFS)
        nbb_init.append((t, nc.sync.dma_start(t[:, 3:27, :], io["nbhi"])))

    def emit_loads(b):
        # ld: 0-1 qd8, 2-3 md8, 4-27 lo_comb(h,kc), 28-29 qT8 (evac space)
        ld = lp.tile([128, 30, Q], f8, tag="ld", name=f"ld_{b}")
        nc.sync.dma_start(ld[:, 0:28, :], io["inT"][b])
        ldb = lp.tile([128, 2, Q], bf, tag="ldb", name=f"ldb_{b}")
        nc.sync.dma_start(ldb[:], io["inTb"][b])
        nbb = pp.tile([128, 27, Q], f8, tag="nbb", name=f"nbb_{b}", bufs=NBB_BUFS)
        nbd = nc.sync.dma_start(nbb[:, 0:3, :], io["bhi"][b])
        if b < NBB_BUFS:
            _adh(nbd.ins, nbb_init[b % NBB_BUFS][1].ins, reason="nbhi resident before bias_hi")
        return ld, ldb, nbb

    def emit_proj(b, ld, ldb, nbb):
        # q/gate: fp8 DoubleRow (evac rescales 1/8); k/v: bf16 (accuracy).
        gt = pp.tile([128, 2, Q], bf, tag="gt", name=f"gt_{b}")
        vv = pp.tile([128, 3, 320], bf, tag="vv", name=f"vv_{b}", bufs=VV_BUFS)
        kTI = pp.tile([128, 25, 128], f8, tag="kTI", name=f"kTI_{b}", bufs=KTI_BUFS)
        if b < KTI_BUFS:
            pass  # zeros + I come from the startup instances on this buffer
        qd = ld[:, 0:2, :]
        md = ld[:, 2:4, :]
        for j in range(2):
            ps = sp.tile([128, 512], f32, tag="sm", name=f"psq{j}_{b}")
            nc.tensor.matmul(
                ps[:, :Q], w_sb["wq"][:, :, 128 * j : 128 * (j + 1)], qd,
                start=True, stop=True, perf_mode=DR,
            )
            # psQ = 8*q ; store qT8 = 4*q in the ld rhs slot
            nc.vector.tensor_scalar_mul(ld[:, 28 + j, :], ps[:, :Q], 0.5)
            ps = sp.tile([128, 512], f32, tag="sm", name=f"psk{j}_{b}")
            nc.tensor.matmul(
                ps[:, :Q], w_sb["wk"][:, :, 128 * j : 128 * (j + 1)], md,
                start=True, stop=True, perf_mode=DR,
            )
            # psK = 8*k ; DVE rescales to k/4 in SBUF, Pool scatters into
            # the zero-padded slabs (GPSIMD cannot read PSUM)
            kq = pp.tile([128, 2, Q], bf, tag="kq", name=f"kq{j}_{b}")
            nc.vector.tensor_scalar_mul(kq[:, j, :], ps[:, :Q], 0.03125)
            for hh in range(4):
                h = 4 * j + hh
                nc.gpsimd.tensor_copy(
                    kTI[32 * hh : 32 * hh + 32, 1 + 3 * h : 4 + 3 * h, :],
                    kq[32 * hh : 32 * hh + 32, j, :].rearrange(
                        "p (c k) -> p c k", k=128
                    ),
                )
            ps = sp.tile([128, 512], f32, tag="sm", name=f"psg{j}_{b}")
            nc.tensor.matmul(
                ps[:, :Q], w_sb["wg"][:, :, 128 * j : 128 * (j + 1)], qd,
                start=True, stop=True, perf_mode=DR,
            )
            nc.scalar.activation(
                gt[:, j, :], ps[:, :Q], Tanh, bias=gbh_sb[:, j : j + 1], scale=0.0625
            )
        for kc in range(3):
            ps = sp.tile([128, 512], f32, tag="sm", name=f"psv{kc}_{b}")
            for a in range(2):
                nc.tensor.matmul(
                    ps[:, :256],
                    ldb[:, a, 128 * kc : 128 * (kc + 1)],
                    w_sb["wv"][:, a, :],
                    start=(a == 0), stop=(a == 1),
                )
            nc.vector.tensor_copy(
                vv[:, kc, 0:264].rearrange("p (h c) -> p h c", c=33)[:, :, 0:32],
                ps[:, :256].rearrange("p (h c) -> p h c", c=32),
            )
        sums_bf = gp.tile([8, Q], bf, tag="sums_bf", name=f"sums_bf_{b}")
        waBIG = wap.tile([128, 4, Q], bf, tag="waBIG", name=f"waBIG_{b}")
        waA = [
            gp.tile([128, Q], bf, tag=f"waA{j}", name=f"waA{j}_{b}") for j in range(2)
        ]
        return dict(
            ld=ld, kTI=kTI, nbb=nbb, gt=gt, vv=vv,
            sums_bf=sums_bf, waBIG=waBIG, waA=waA,
            wa_evac=[None] * 4, psW=None,
        )

    def emit_heads(b, st, first=False):
        ld, kTI, nbb = st["ld"], st["kTI"], st["nbb"]
        vv, waBIG = st["vv"], st["waBIG"]
        for h in range(8):
            j, hh, p, pw = h // 4, h % 4, h // 2, h % 2
            psS = Sp.tile([128, 1536], f32, tag="S", name=f"psS{h}_{b}")
            for kc in range(3):
                s = 3 * h + kc
                # DR1: psS = I*lo_comb + slab.T*qT8  (= lo + S^T)
                sm = nc.tensor.matmul(
                    psS[:, 512 * kc : 512 * kc + Q],
                    kTI[:, 0 : 2 + s : 1 + s, :],
                    ld[:, 4 + s : 29 + j : 24 + j - s, :],
                    start=True, stop=False, perf_mode=DR,
                )
                if first and h == 0:
                    _adh(sm.ins, kti_init[0][1].ins, reason="I before first S-DR")
                # DR2: psS += bias_hi(kc) + nb_hi(h,kc)
                fm = nc.tensor.matmul(
                    psS[:, 512 * kc : 512 * kc + Q],
                    id2[:],
                    nbb[:, kc : 4 + s : 3 + 3 * h, :],
                    start=False, stop=True, perf_mode=DR,
                )
                if first and h == 0:
                    _adh(fm.ins, id2_dma.ins, reason="id2 load before first fold")
            sview = psS[:].rearrange("p (c x) -> p c x", x=512)[:, :, :Q]
            w4 = wp.tile([128, 3, Q], bf, tag="w4", name=f"w4_{h}_{b}")
            nc.scalar.activation(w4[:], sview, Exp, bias=shift_sb[:])
            if pw == 0:
                st["psW"] = sp.tile([128, 512], f32, tag="sm", name=f"psW{h}_{b}")
            psW = st["psW"]
            for kc in range(3):
                nc.tensor.matmul(
                    psW[64 * pw : 64 * pw + 64, :Q],
                    vv[:, kc, 33 * h : 33 * h + 64],
                    w4[:, kc, :],
                    start=(kc == 0), stop=(kc == 2),
                )
            if pw == 1:
                ev = nc.vector.tensor_copy(waBIG[:, p, :], psW[:, :Q])
                st["wa_evac"][p] = ev

    # raw-AP gather bookkeeping for WAR insurance on waBIG reuse
    last_gathers = {}

    def emit_tail(b, st):
        waBIG, sums_bf, waA, gt = st["waBIG"], st["sums_bf"], st["waA"], st["gt"]
        evacs = st["wa_evac"]
        parity = b % int(_env("WA_BUFS", "4"))
        # one DMA: the 8 denominator rows (2*sum) -> sums_bf [8, Q].
        # Row order is (r, p): row i holds head 2*(i%4) + i//4; the host ind
        # matrix is permuted to match.
        src = _br.AP(
            waBIG.tensor,
            waBIG.offset + 32 * 4 * Q,
            [[64 * 4 * Q, 2], [Q, 4], [1, Q]],
        )
        dst = _br.AP(sums_bf.tensor, sums_bf.offset, [[Q, 8], [1, Q]])
        d = nc.sync.dma_start(dst, src)
        for ev in evacs:
            _adh(d.ins, ev.ins, reason="sums gather reads waBIG")
        gathers = [d]
        # cast + reciprocal + bf16 rebroadcast source
        sums_f = gp.tile([8, Q], f32, tag="sums_f", name=f"sums_f_{b}")
        sc = nc.gpsimd.tensor_copy(sums_f[:], sums_bf[:])
        _adh(sc.ins, d.ins, reason="sums_bf filled by gather")
        rec = gp.tile([8, Q], f32, tag="rec", name=f"rec_{b}")
        nc.vector.reciprocal_approx_fast(rec[:], sums_f[:])
        recb = gp.tile([8, Q], bf, tag="recb", name=f"recb_{b}")
        nc.gpsimd.tensor_copy(recb[:], rec[:])
        # head blocks into gate-aligned waA tiles (tile-slice DMAs; partition
        # shifts are DMA-only).  head h: waBIG[64*(h%2):+32, h//2] ->
        # waA[h//4][32*(h%4):+32]
        wadmas = []
        for h in range(8):
            j, hh, p, r = h // 4, h % 4, h // 2, h % 2
            dj = nc.sync.dma_start(
                waA[j][32 * hh : 32 * hh + 32, :],
                waBIG[64 * r : 64 * r + 32, p, :],
            )
            wadmas.append(dj)
            gathers.append(dj)
        last_gathers[parity] = gathers

        ga_tiles = []
        for j in range(2):
            psR = sp.tile([128, 512], f32, tag="sm", name=f"psR{j}_{b}")
            nc.tensor.matmul(
                psR[:, :Q], ind_sb[0:8, 128 * j : 128 * (j + 1)], recb[:],
                start=True, stop=True,
            )
            g2 = gp.tile([128, Q], bf, tag="g2", name=f"g2{j}_{b}")
            nc.vector.scalar_tensor_tensor(
                g2[:], gt[:, j, :], 1.0, psR[:, :Q], op0=ADD, op1=MUL
            )
            ga = gp.tile([128, Q], bf, tag="ga", name=f"ga{j}_{b}")
            gm = nc.gpsimd.tensor_tensor(ga[:], waA[j][:], g2[:], op=MUL)
            ga_tiles.append(ga)
        ob = outp.tile([128, 3, OUT], f32, tag="ob", name=f"ob_{b}")
        for qc in range(3):
            psO = sp.tile([128, 512], f32, tag="sm", name=f"psO{qc}_{b}")
            for j in range(2):
                nc.tensor.matmul(
                    psO[:, :OUT],
                    ga_tiles[j][:, 128 * qc : 128 * (qc + 1)],
                    w_sb["wo"][:, j, :],
                    start=(j == 0), stop=False,
                )
            nc.tensor.matmul(
                psO[:, :OUT], ones_row[:], obias_row[:], start=False, stop=True
            )
            nc.vector.tensor_copy(ob[:, qc, :], psO[:, :OUT])
        nc.sync.dma_start(io["out"][b].rearrange("(c p) o -> p c o", p=128), ob[:])

    def guard_evacs(st, b):
        # WAR insurance: this batch's waBIG writes wait for the gathers that
        # read the buffer two batches ago (raw-AP reads are invisible to the
        # tile tracker).
        parity = b % int(_env("WA_BUFS", "4"))
        old = last_gathers.get(parity)
        if old:
            for ev in st["wa_evac"]:
                for g in old:
                    _adh(ev.ins, g.ins, reason="waBIG reuse after raw gather")

    # Two-batch software pipeline.  Emission (= per-engine execution) order
    # per iteration: heads(b), proj(b+2), tail(b-1).  The tail's PE matmuls
    # thus trail a full batch behind the DVE/Pool chains that feed them, so
    # they never stall PE in front of the next batch's S matmuls, and ACT
    # chains exp(b) -> tanh(b+2) -> exp(b+1) without holes.
    def prep(b):
        ld, ldb, nbb = emit_loads(b)
        return emit_proj(b, ld, ldb, nbb)

    states = {0: prep(0)}
    if bpc > 1:
        states[1] = prep(1)
    done = {}
    for b in range(bpc):
        st = states.pop(b)
        emit_heads(b, st, first=(b == 0))
        guard_evacs(st, b)
        done[b] = st
        if b + 2 < bpc:
            states[b + 2] = prep(b + 2)
        if b >= 1:
            emit_tail(b - 1, done.pop(b - 1))
    emit_tail(bpc - 1, done.pop(bpc - 1))


def build(bpc=BPC):
    nc = bacc.Bacc(
        "TRN2",
        target_bir_lowering=False,
        debug=False,
        enable_asserts=False,
        num_devices=NUM_CORES,
    )
    f32, bf, f8 = mybir.dt.float32, mybir.dt.bfloat16, mybir.dt.float8e4
    io = {
        "inT": nc.dram_tensor("inT", [bpc, 128, 28, Q], f8, kind="ExternalInput").ap(),
        "inTb": nc.dram_tensor("inTb", [bpc, 128, 2, Q], bf, kind="ExternalInput").ap(),
        "wq": nc.dram_tensor("wq", [128, 2, 256], f8, kind="ExternalInput").ap(),
        "wk": nc.dram_tensor("wk", [128, 2, 256], f8, kind="ExternalInput").ap(),
        "wv": nc.dram_tensor("wv", [128, 2, 256], bf, kind="ExternalInput").ap(),
        "wg": nc.dram_tensor("wg", [128, 2, 256], f8, kind="ExternalInput").ap(),
        "wo": nc.dram_tensor("wo", [128, 2, 256], bf, kind="ExternalInput").ap(),
        "obias_bf": nc.dram_tensor("obias_bf", [1, OUT], bf, kind="ExternalInput").ap(),
        "gbh": nc.dram_tensor("gbh", [128, 2], f32, kind="ExternalInput").ap(),
        "ind": nc.dram_tensor("ind", [128, 256], bf, kind="ExternalInput").ap(),
        "id2": nc.dram_tensor("id2", [128, 2, 128], f8, kind="ExternalInput").ap(),
        "nbhi": nc.dram_tensor("nbhi", [128, 24, Q], f8, kind="ExternalInput").ap(),
        "bhi": nc.dram_tensor("bhi", [bpc, 128, 3, Q], f8, kind="ExternalInput").ap(),
        "out": nc.dram_tensor("out", [bpc, Q, OUT], f32, kind="ExternalOutput").ap(),
    }
    with tile.TileContext(nc) as tc:
        with ExitStack() as ctx:
            _build_body(ctx, tc, io, bpc)
    nc.compile()
    return nc


def _prep_inputs(
    q_data,
    m_data,
    bias,
    nonbatched_bias,
    q_weights,
    k_weights,
    v_weights,
    o_weights,
    o_bias,
    gating_w,
    gating_b,
):
    """Host-side preprocessing into the DMA-friendly device layouts."""
    scale = q_weights.shape[-1] ** -0.5

    def featT(x, s):  # [B, S, A] -> [B, 128, A//128, S] scaled
        b, sl, a = x.shape
        t = x.transpose(0, 2, 1).reshape(b, a // 128, 128, sl).transpose(0, 2, 1, 3)
        return np.ascontiguousarray((t * s).astype(F8))

    qdT = featT(q_data, 0.125)  # [B, 128, 2, Q]

    def featTb(x):  # [B, S, A] -> [B, 128, A//128, S] bf16
        b, sl, a = x.shape
        t = x.transpose(0, 2, 1).reshape(b, a // 128, 128, sl).transpose(0, 2, 1, 3)
        return np.ascontiguousarray(t.astype(BF16))

    mdTb = featTb(m_data)

    # fused[b, h] = (bias[b] + nb[h])^T in chunk layout [128, 8, 3, Q]
    biasT = bias[:, 0].transpose(0, 2, 1).astype(np.float32)  # [B, K, Q]
    nbT = nonbatched_bias.transpose(0, 2, 1).astype(np.float32)  # [H, K, Q]
    mdT8 = featT(m_data, 0.125)
    inT = np.empty((B, 128, 28, Q), dtype=F8)
    inT[:, :, 0:2, :] = qdT
    inT[:, :, 2:4, :] = mdT8
    # coarse halves: 0.5-granular, exactly representable in fp8e4
    nbhi_f = np.clip(np.round(nbT * 2.0) / 2.0, -8.0, 8.0)  # [H, K, Q]
    bhi_f = np.clip(np.round(biasT * 2.0) / 2.0, -8.0, 8.0)  # [B, K, Q]
    nbhi = np.ascontiguousarray(
        nbhi_f.reshape(H, 3, 128, Q).transpose(2, 0, 1, 3).reshape(128, 24, Q)
    ).astype(F8)
    bhi = np.ascontiguousarray(
        bhi_f.reshape(B, 3, 128, Q).transpose(0, 2, 1, 3)
    ).astype(F8)
    for b in range(B):
        lo = (biasT[b] - bhi_f[b])[None] + (nbT - nbhi_f)  # [H, K, Q]
        fc = lo.reshape(H, 3, 128, Q).transpose(2, 0, 1, 3)
        inT[b, :, 4:28, :] = fc.reshape(128, 24, Q).astype(F8)

    def wmat(w, s):  # [A, H, hd] -> [128, 2, 256]
        m = (w.reshape(A, H * HD) * s).astype(F8)
        return np.ascontiguousarray(m.reshape(2, 128, 256).transpose(1, 0, 2))

    def wmatb(w):  # [A, H, hd] -> [128, 2, 256] bf16
        m = w.reshape(A, H * HD).astype(BF16)
        return np.ascontiguousarray(m.reshape(2, 128, 256).transpose(1, 0, 2))

    wq = wmat(q_weights, 64.0 * scale)
    wk = wmat(k_weights, 64.0)
    wv = wmatb(v_weights)
    wg = wmat(gating_w, 64.0)
    wo = np.ascontiguousarray(
        o_weights.reshape(256, 256).astype(BF16).reshape(2, 128, 256).transpose(1, 0, 2)
    )
    obias_bf = np.ascontiguousarray(o_bias.astype(BF16).reshape(1, OUT))
    gbh = np.ascontiguousarray(
        (0.5 * gating_b.reshape(H * HD).astype(np.float32)).reshape(2, 128).T
    )
    ind = np.zeros((128, 256), dtype=BF16)
    # sums_bf row i holds head 2*(i%4) + i//4 (gather iterates (r, p))
    for i in range(8):
        h = 2 * (i % 4) + i // 4
        ind[i, 32 * h : 32 * (h + 1)] = 1.0
    ind[127, :] = 1.0  # ones row for the o_bias rank-1 matmul
    id2 = np.zeros((128, 2, 128), dtype=F8)
    id2[:, 0, :] = np.eye(128, dtype=np.float32).astype(F8)
    id2[:, 1, :] = id2[:, 0, :]
    return dict(
        inT=inT, inTb=mdTb, wq=wq, wk=wk, wv=wv, wg=wg, wo=wo,
        obias_bf=obias_bf, gbh=gbh, ind=ind, id2=id2, nbhi=nbhi, bhi=bhi,
    )


_NC_CACHE = {}


def kernel(**inputs):
    from concourse.bass_utils import run_bass_kernel_spmd

    full = _prep_inputs(**{k: np.asarray(v) for k, v in inputs.items()})
    if BPC not in _NC_CACHE:
        _NC_CACHE[BPC] = build(BPC)
    nc = _NC_CACHE[BPC]

    shared = {
        k: full[k]
        for k in ("wq", "wk", "wv", "wg", "wo", "obias_bf", "gbh", "ind", "id2", "nbhi")
    }
    in_maps = []
    for c in range(NUM_CORES):
        sl = slice(c * BPC, (c + 1) * BPC)
        in_maps.append(dict(inT=full["inT"][sl], inTb=full["inTb"][sl], bhi=full["bhi"][sl], **shared))

    trace = bool(int(os.environ.get("BASS_KERNEL_TRACE", "0")))
    if trace:
        try:
            from antenv.axon_hooks import get_axon_ntff_profile_hook  # noqa: F401
        except Exception:
            trace = False
    import time

    t0 = time.time()
    res = run_bass_kernel_spmd(
        nc, in_maps, core_ids=list(range(NUM_CORES)), trace=trace
    )
    kernel.last_run_wall_s = time.time() - t0
    if trace and res.exec_time_ns is not None:
        print(f"HW exec time: {res.exec_time_ns} ns")
        kernel.last_exec_time_ns = res.exec_time_ns
    out = np.concatenate([r["out"] for r in res.results], axis=0)
    return out.astype(np.float32)
